# revision 21
# baseline (speedup 1.0000x reference)
"""Trainium2 Bass kernel for the DCT-CNN expert core.

Reference computation (per 512x512 single-channel image):
  1. split into 4096 non-overlapping 8x8 patches
  2. 2D DCT per patch:  c = D @ p @ D^T
  3. conv3x3(1->16, SAME) + bias + relu on each 8x8 patch image
  4. conv3x3(16->32, SAME) + bias
  5. mean over spatial (8x8), then mean over patches  -> [B, 32]

Algebraic restructuring (validated to fp32 roundoff):
  - DCT + conv1 fold into one [1024, 64] matrix W = M1 @ (D (x) D); with bias
    b1 broadcast per channel:  h1 = relu(W @ p + b1h)
  - conv2 + spatial mean + patch mean fold into a [1024, 32] matrix applied
    to the per-image SUM of h1: out[b] = (sum_p h1)^T @ M2e + b2

Device schedule per core (2 images = 8192 patches):
  - 64 relu+bias+accumulate tiles of [128, 1024] f32 PSUM (2 banks each,
    3 in flight), split between ScalarE activation and VectorE
    tensor_scalar (both engines read PSUM at 1 elem/cycle/lane; TRN2
    matmul cannot write 16-bit PSUM, so 2x DVE modes are unreachable).
  - PE warmup matmuls on dummy data beat the HAM clock gate before the
    input DMAs land; weights are a single [64, 1024] copy shared by both
    PE row groups via explicit tile_position.
  - final [128,2]x[128,32] matmuls accumulate per-k inside the main loop
    (GPSIMD sums the per-quarter accumulators), leaving a short tail.

Sharding: pure data parallel over images (2 per core), weights replicated.
"""
import numpy as np

import concourse.bass as bass
import concourse.bacc as bacc
import concourse.tile as tile
from concourse import mybir
from concourse.bass_utils import run_bass_kernel_spmd

N_CORES = 8
F32 = mybir.dt.float32
BF16 = mybir.dt.bfloat16

try:
    import ml_dtypes
    NP_BF16 = np.dtype(ml_dtypes.bfloat16)
except ImportError:  # pragma: no cover
    NP_BF16 = None

# ---------------------------------------------------------------- host math

def _dct_matrix(n=8):
    m = np.zeros((n, n), dtype=np.float64)
    for k in range(n):
        for t in range(n):
            if k == 0:
                m[k, t] = 1.0 / np.sqrt(n)
            else:
                m[k, t] = np.sqrt(2.0 / n) * np.cos(np.pi * k * (2 * t + 1) / (2.0 * n))
    return m


def _conv3x3_matrix(w):
    """Dense linear operator of a SAME 3x3 cross-correlation on 8x8 images.

    w: [O, I, 3, 3] -> M: [O*64, I*64] with
    flatten(conv(img))[(o,y,x)] = sum M[(o,y,x),(i,r,c)] img[i,r,c]
    """
    O, I = w.shape[0], w.shape[1]
    M = np.zeros((O, 8, 8, I, 8, 8))
    for dy in range(3):
        for dx in range(3):
            ylo, yhi = max(0, 1 - dy), min(8, 9 - dy)
            xlo, xhi = max(0, 1 - dx), min(8, 9 - dx)
            for y in range(ylo, yhi):
                for x in range(xlo, xhi):
                    M[:, y, x, :, y + dy - 1, x + dx - 1] += w[:, :, dy, dx]
    return M.reshape(O * 64, I * 64)


def _build_weights(w1, b1, w2, b2):
    """Returns (Wt [64,1024], b1c [128,8], M2c [128,256], b2t [128,32]) f32."""
    D = _dct_matrix()
    KRON = np.kron(D, D)                                   # c_flat = KRON @ p_flat
    M1 = _conv3x3_matrix(w1.astype(np.float64))            # [1024, 64]
    M1K = M1 @ KRON                                        # [1024, 64]
    b1h = np.repeat(b1.astype(np.float64), 64)             # [1024]
    M2 = _conv3x3_matrix(w2.astype(np.float64))            # [2048, 1024]
    A2 = M2.reshape(32, 64, 1024).sum(axis=1)              # [32, 1024]
    M2e = A2.T / (64.0 * 4096.0)                           # [1024, 32]

    Wt = np.ascontiguousarray(M1K.T, dtype=np.float32)     # [64, 1024]
    b1c = np.ascontiguousarray(
        b1h.reshape(8, 128).T, dtype=np.float32)           # [128, 8]
    M2c = np.ascontiguousarray(
        M2e.reshape(8, 128, 32).transpose(1, 0, 2).reshape(128, 256),
        dtype=np.float32)                                  # [128, 8*32]
    b2t = np.ascontiguousarray(
        np.tile(b2.astype(np.float32), (128, 1)))          # [128, 32]
    return Wt, b1c, M2c, b2t


# ------------------------------------------------------------- device kernel

# aux1 (f32 columns): [0:8) b1 chunks (col k = b1h[128k:128k+128]);
#                     [8:40) b2 broadcast to all partitions.
# aux2 (f32): M2e chunks (cols 32k..32k+32 = M2e[128k:128k+128, :]).
AUXB1 = 0
AUXB2 = 8

# relu engine assignment: ~36/64 tiles on ScalarE (ACT), rest on VectorE.
_N_TILES = 64
_ACT_SHARE = 36
_N_WARM = 5  # dummy PE warmup matmuls (HAM clock-gate ramp)


def _build_nc():
    nc = bacc.Bacc("TRN2", target_bir_lowering=False, debug=False,
                   num_devices=N_CORES)
    p_d = nc.declare_dram_parameter("p", [128, 4096], BF16, isOutput=False)
    wts_d = nc.declare_dram_parameter("wts", [128, 1024], BF16, isOutput=False)
    aux1_d = nc.declare_dram_parameter("aux1", [128, 40], F32, isOutput=False)
    aux2_d = nc.declare_dram_parameter("aux2", [128, 256], F32, isOutput=False)
    out_d = nc.declare_dram_parameter("out", [2, 32], F32, isOutput=True)

    act_flags = [(((i + 1) * _ACT_SHARE) // _N_TILES) > ((i * _ACT_SHARE) // _N_TILES)
                 for i in range(_N_TILES)]

    with tile.TileContext(nc) as tc:
        with (
            tc.tile_pool(name="persist", bufs=1) as persist,
            tc.tile_pool(name="psum", bufs=1, space="PSUM") as psum,
        ):
            # --- ACT table preload (Relu) on a tiny dummy, ahead of data.
            dum_a = persist.tile([128, 8], F32)
            nc.vector.memset(dum_a, 0.0)
            nc.scalar.activation(dum_a, dum_a,
                                 mybir.ActivationFunctionType.Relu,
                                 bias=0.0, scale=1.0)

            # --- PE warmup: dummy matmuls to ramp the HAM clock gate while
            # the input DMAs are in flight.
            wdum = persist.tile([64, 128], BF16)
            rdum = persist.tile([64, 512], BF16)
            zeros_t = persist.tile([128, 1], F32)
            nc.vector.memset(wdum, 0.0)
            nc.vector.memset(rdum, 0.0)
            nc.vector.memset(zeros_t, 0.0)
            warm_ps = psum.tile([128, 1024], F32, tag="ps", bufs=4,
                                name="warm_ps")
            for _ in range(_N_WARM):
                nc.tensor.matmul(warm_ps[:, 0:512], lhsT=wdum, rhs=rdum,
                                 start=True, stop=True, tile_position=(0, 0))

            # --- input DMAs, ordered so the first matmul's data lands first.
            wts_t = persist.tile([128, 1024], BF16)
            nc.sync.dma_start(out=wts_t, in_=wts_d[:, :])
            p_t = persist.tile([128, 4096], BF16)
            nc.gpsimd.dma_start(out=p_t[:, 0:1024], in_=p_d[:, 0:1024])
            nc.sync.dma_start(out=p_t[:, 1024:2048], in_=p_d[:, 1024:2048])
            aux1_t = persist.tile([128, 40], F32)
            nc.gpsimd.dma_start(out=aux1_t, in_=aux1_d[:, :])
            nc.sync.dma_start(out=p_t[:, 2048:3072], in_=p_d[:, 2048:3072])
            aux2_t = persist.tile([128, 256], F32)
            nc.gpsimd.dma_start(out=p_t[:, 3072:4096], in_=p_d[:, 3072:4096])
            nc.sync.dma_start(out=aux2_t, in_=aux2_d[:, :])

            acc_t = persist.tile([128, 64], F32)
            t4_t = persist.tile([128, 4], F32)
            s_t = persist.tile([128, 16], F32)
            fin_sb = persist.tile([2, 32], F32)

            # Main loop. Per (k, quarter): one f32 PSUM tile per image, the
            # two tiles' matmuls interleaved so consecutive MMs target
            # different PE row groups (both sub-array halves run
            # concurrently). relu+bias+accumulate drains each tile on
            # ScalarE or VectorE per act_flags; accumulators land in
            # acc_t[:, 8k+2q+img].
            ti = 0
            for k in range(8):
                b1_ap = aux1_t[:, AUXB1 + k:AUXB1 + k + 1]
                for q in range(4):
                    tiles = []
                    for img in range(2):
                        tiles.append(psum.tile([128, 1024], F32, tag="ps",
                                               bufs=4, name=f"ps_{k}_{q}_{img}"))
                    for j in range(2):
                        for img in range(2):
                            c0 = 1024 * q + 512 * j
                            nc.tensor.matmul(
                                tiles[img][:, 512 * j:512 * j + 512],
                                lhsT=wts_t[64 * img:64 * img + 64,
                                           128 * k:128 * k + 128],
                                rhs=p_t[64 * img:64 * img + 64, c0:c0 + 512],
                                start=True, stop=True,
                            )
                    for img in range(2):
                        col = 8 * k + 2 * q + img
                        accap = acc_t[:, col:col + 1]
                        if act_flags[ti]:
                            nc.scalar.activation(
                                tiles[img], tiles[img],
                                mybir.ActivationFunctionType.Relu,
                                bias=b1_ap, scale=1.0, accum_out=accap,
                            )
                        else:
                            # out = max(in + b1, 0); accum_out = sum(out).
                            # (tensor_scalar would be cheaper but its
                            # accumulator command is LOAD_ACCUMULATE — a
                            # running sum across ops — so the per-tile
                            # sums come out wrong; STT uses
                            # ZERO_ACCUMULATE.)
                            nc.vector.scalar_tensor_tensor(
                                out=tiles[img], in0=tiles[img],
                                scalar=b1_ap,
                                in1=zeros_t.to_broadcast([128, 1024]),
                                op0=mybir.AluOpType.add,
                                op1=mybir.AluOpType.max,
                                accum_out=accap,
                            )
                        ti += 1

                # per-k tail, overlapped with the next k's main work:
                # s[:, 2k+img] = sum_q acc[:, 8k+2q+img] via two contiguous
                # pairwise adds on the otherwise-idle GPSIMD engine, then
                # ps_f[0:2] += s_k^T @ M2e_k
                nc.gpsimd.tensor_tensor(
                    out=t4_t, in0=acc_t[:, 8 * k:8 * k + 4],
                    in1=acc_t[:, 8 * k + 4:8 * k + 8],
                    op=mybir.AluOpType.add,
                )
                nc.gpsimd.tensor_tensor(
                    out=s_t[:, 2 * k:2 * k + 2], in0=t4_t[:, 0:2],
                    in1=t4_t[:, 2:4],
                    op=mybir.AluOpType.add,
                )
                ps_k = psum.tile([128, 1024], F32, tag="ps", bufs=4,
                                 name=f"ps_fin_{k}")
                nc.tensor.matmul(
                    ps_k[0:2, 0:32],
                    lhsT=s_t[:, 2 * k:2 * k + 2],
                    rhs=aux2_t[:, 32 * k:32 * k + 32],
                    start=True, stop=True,
                )
                # fin_sb accumulates the per-k contributions (seeded with b2)
                nc.vector.tensor_tensor(
                    out=fin_sb, in0=ps_k[0:2, 0:32],
                    in1=(aux1_t[0:2, AUXB2:AUXB2 + 32] if k == 0 else fin_sb),
                    op=mybir.AluOpType.add,
                )

            nc.sync.dma_start(out=out_d[:, :], in_=fin_sb)

    nc.compile()
    return nc


_NC_CACHE = None
TRACE = False
_last_result = None
_last_profile_dir = None


def _get_nc():
    global _NC_CACHE
    if _NC_CACHE is None:
        _NC_CACHE = _build_nc()
    return _NC_CACHE


def kernel(x, w1, b1, w2, b2):
    global _last_result
    x = np.ascontiguousarray(np.asarray(x, dtype=np.float32))
    Wt, b1c, M2c, b2t = _build_weights(
        np.asarray(w1, np.float32), np.asarray(b1, np.float32),
        np.asarray(w2, np.float32), np.asarray(b2, np.float32))

    wts = np.empty((128, 1024), dtype=NP_BF16)             # W on both halves
    wts[0:64] = Wt.astype(NP_BF16)
    wts[64:128] = wts[0:64]
    aux1 = np.empty((128, 40), dtype=np.float32)
    aux1[:, AUXB1:AUXB1 + 8] = b1c
    aux1[:, AUXB2:AUXB2 + 32] = b2t
    aux2 = np.ascontiguousarray(M2c)

    # patches: x [16,1,512,512] -> [b, pixel(r,c), patch(i,j)] = [16, 64, 4096]
    p_all = (x.reshape(16, 64, 8, 64, 8).transpose(0, 2, 4, 1, 3)
             .reshape(16, 64, 4096).astype(NP_BF16))

    in_maps = []
    for c in range(N_CORES):
        pc = np.empty((128, 4096), dtype=NP_BF16)
        pc[0:64] = p_all[2 * c]
        pc[64:128] = p_all[2 * c + 1]
        in_maps.append({"p": pc, "wts": wts, "aux1": aux1, "aux2": aux2})

    nc = _get_nc()
    if TRACE:
        # Local profiling path: NTFF via direct ctypes calls into the
        # injected libaxon_pjrt.so (this container's antenv stub lacks
        # axon_hooks, so the registered-hook path is unavailable).
        import contextlib
        import ctypes
        import tempfile
        from concourse import bass2jax

        lib = ctypes.CDLL("/opt/axon/libaxon_pjrt.so")
        lib.axon_start_nrt_profile.argtypes = [
            ctypes.POINTER(ctypes.c_int64), ctypes.c_size_t]
        lib.axon_start_nrt_profile.restype = ctypes.c_int64
        lib.axon_stop_nrt_profile.argtypes = [ctypes.c_char_p]
        lib.axon_stop_nrt_profile.restype = ctypes.c_int64

        @contextlib.contextmanager
        def hook(output_dir, device_ids):
            import jax
            jax.devices()
            if device_ids:
                ids = (ctypes.c_int64 * len(device_ids))(*device_ids)
                rc = lib.axon_start_nrt_profile(ids, len(device_ids))
            else:
                rc = lib.axon_start_nrt_profile(None, 0)
            if rc != 0:
                raise RuntimeError(f"axon_start_nrt_profile rc={rc}")
            try:
                yield
            finally:
                n = lib.axon_stop_nrt_profile(str(output_dir).encode())
                print(f"profile: {n} file(s) written to {output_dir}")

        global _last_profile_dir
        tmpdir = tempfile.mkdtemp(prefix="dctcnn_prof_")
        with hook(tmpdir, [0]):
            results = bass2jax.run_bass_via_pjrt(nc, in_maps, n_cores=N_CORES)
        _last_profile_dir = tmpdir
        out = np.concatenate([results[c]["out"] for c in range(N_CORES)], axis=0)
        return out.astype(np.float32)
    res = run_bass_kernel_spmd(nc, in_maps, list(range(N_CORES)))
    _last_result = res
    out = np.concatenate([res.results[c]["out"] for c in range(N_CORES)], axis=0)
    return out.astype(np.float32)


# revision 24
# speedup vs baseline: 1.0771x; 1.0771x over previous
"""Trainium2 Bass kernel for the DCT-CNN expert core.

Reference computation (per 512x512 single-channel image):
  1. split into 4096 non-overlapping 8x8 patches
  2. 2D DCT per patch:  c = D @ p @ D^T
  3. conv3x3(1->16, SAME) + bias + relu on each 8x8 patch image
  4. conv3x3(16->32, SAME) + bias
  5. mean over spatial (8x8), then mean over patches  -> [B, 32]

Algebraic restructuring (validated to fp32 roundoff):
  - DCT + conv1 fold into one [1024, 64] matrix W = M1 @ (D (x) D); with bias
    b1 broadcast per channel:  h1 = relu(W @ p + b1h)
  - conv2 + spatial mean + patch mean fold into a [1024, 32] matrix applied
    to the per-image SUM of h1: out[b] = (sum_p h1)^T @ M2e + b2

Device schedule per core (2 images = 8192 patches):
  - 64 relu+bias+accumulate tiles of [128, 1024] f32 PSUM (2 banks each,
    3 in flight), split between ScalarE activation and VectorE
    tensor_scalar (both engines read PSUM at 1 elem/cycle/lane; TRN2
    matmul cannot write 16-bit PSUM, so 2x DVE modes are unreachable).
  - PE warmup matmuls on dummy data beat the HAM clock gate before the
    input DMAs land; weights are a single [64, 1024] copy shared by both
    PE row groups via explicit tile_position.
  - final [128,2]x[128,32] matmuls accumulate per-k inside the main loop
    (GPSIMD sums the per-quarter accumulators), leaving a short tail.

Sharding: pure data parallel over images (2 per core), weights replicated.
"""
import numpy as np

import concourse.bass as bass
import concourse.bacc as bacc
import concourse.tile as tile
from concourse import mybir
from concourse.bass_utils import run_bass_kernel_spmd

N_CORES = 8
F32 = mybir.dt.float32
BF16 = mybir.dt.bfloat16

try:
    import ml_dtypes
    NP_BF16 = np.dtype(ml_dtypes.bfloat16)
except ImportError:  # pragma: no cover
    NP_BF16 = None

# ---------------------------------------------------------------- host math

def _dct_matrix(n=8):
    m = np.zeros((n, n), dtype=np.float64)
    for k in range(n):
        for t in range(n):
            if k == 0:
                m[k, t] = 1.0 / np.sqrt(n)
            else:
                m[k, t] = np.sqrt(2.0 / n) * np.cos(np.pi * k * (2 * t + 1) / (2.0 * n))
    return m


def _conv3x3_matrix(w):
    """Dense linear operator of a SAME 3x3 cross-correlation on 8x8 images.

    w: [O, I, 3, 3] -> M: [O*64, I*64] with
    flatten(conv(img))[(o,y,x)] = sum M[(o,y,x),(i,r,c)] img[i,r,c]
    """
    O, I = w.shape[0], w.shape[1]
    M = np.zeros((O, 8, 8, I, 8, 8))
    for dy in range(3):
        for dx in range(3):
            ylo, yhi = max(0, 1 - dy), min(8, 9 - dy)
            xlo, xhi = max(0, 1 - dx), min(8, 9 - dx)
            for y in range(ylo, yhi):
                for x in range(xlo, xhi):
                    M[:, y, x, :, y + dy - 1, x + dx - 1] += w[:, :, dy, dx]
    return M.reshape(O * 64, I * 64)


def _build_weights(w1, b1, w2, b2):
    """Returns (Wt [64,1024], b1c [128,8], M2c [128,256], b2t [128,32]) f32."""
    D = _dct_matrix()
    KRON = np.kron(D, D)                                   # c_flat = KRON @ p_flat
    M1 = _conv3x3_matrix(w1.astype(np.float64))            # [1024, 64]
    M1K = M1 @ KRON                                        # [1024, 64]
    b1h = np.repeat(b1.astype(np.float64), 64)             # [1024]
    M2 = _conv3x3_matrix(w2.astype(np.float64))            # [2048, 1024]
    A2 = M2.reshape(32, 64, 1024).sum(axis=1)              # [32, 1024]
    M2e = A2.T / (64.0 * 4096.0)                           # [1024, 32]

    Wt = np.ascontiguousarray(M1K.T, dtype=np.float32)     # [64, 1024]
    b1c = np.ascontiguousarray(
        b1h.reshape(8, 128).T, dtype=np.float32)           # [128, 8]
    M2c = np.ascontiguousarray(
        M2e.reshape(8, 128, 32).transpose(1, 0, 2).reshape(128, 256),
        dtype=np.float32)                                  # [128, 8*32]
    b2t = np.ascontiguousarray(
        np.tile(b2.astype(np.float32), (128, 1)))          # [128, 32]
    return Wt, b1c, M2c, b2t


# ------------------------------------------------------------- device kernel

# aux1 (f32 columns): [0:8) b1 chunks (col k = b1h[128k:128k+128]);
#                     [8:40) b2 broadcast to all partitions.
# aux2 (f32): M2e chunks (cols 32k..32k+32 = M2e[128k:128k+128, :]).
AUXB1 = 0
AUXB2 = 8

# relu engine assignment: ~33/64 tiles on ScalarE (ACT), rest on VectorE.
_N_TILES = 64
_ACT_SHARE = 33
_N_WARM = 5  # dummy PE warmup matmuls (HAM clock-gate ramp)


def _build_nc():
    nc = bacc.Bacc("TRN2", target_bir_lowering=False, debug=False,
                   num_devices=N_CORES)
    p_d = nc.declare_dram_parameter("p", [128, 4096], BF16, isOutput=False)
    wts_d = nc.declare_dram_parameter("wts", [128, 1024], BF16, isOutput=False)
    aux1_d = nc.declare_dram_parameter("aux1", [128, 40], F32, isOutput=False)
    aux2_d = nc.declare_dram_parameter("aux2", [128, 256], F32, isOutput=False)
    out_d = nc.declare_dram_parameter("out", [2, 32], F32, isOutput=True)

    act_flags = [(((i + 1) * _ACT_SHARE) // _N_TILES) > ((i * _ACT_SHARE) // _N_TILES)
                 for i in range(_N_TILES)]

    with tile.TileContext(nc) as tc:
        with (
            tc.tile_pool(name="persist", bufs=1) as persist,
            tc.tile_pool(name="psum", bufs=1, space="PSUM") as psum,
        ):
            # --- ACT table preload (Relu) on a tiny dummy, ahead of data.
            dum_a = persist.tile([128, 8], F32)
            nc.vector.memset(dum_a, 0.0)
            nc.scalar.activation(dum_a, dum_a,
                                 mybir.ActivationFunctionType.Relu,
                                 bias=0.0, scale=1.0)

            # --- PE warmup: dummy matmuls to ramp the HAM clock gate while
            # the input DMAs are in flight.
            wdum = persist.tile([64, 128], BF16)
            rdum = persist.tile([64, 512], BF16)
            zeros_t = persist.tile([128, 1], F32)
            nc.vector.memset(wdum, 0.0)
            nc.vector.memset(rdum, 0.0)
            nc.vector.memset(zeros_t, 0.0)
            warm_ps = psum.tile([128, 1024], F32, tag="ps", bufs=4,
                                name="warm_ps")
            for _ in range(_N_WARM):
                nc.tensor.matmul(warm_ps[:, 0:512], lhsT=wdum, rhs=rdum,
                                 start=True, stop=True, tile_position=(0, 0))

            # --- input DMAs, ordered so the first matmul's data lands first.
            wts_t = persist.tile([128, 1024], BF16)
            nc.sync.dma_start(out=wts_t, in_=wts_d[:, :])
            p_t = persist.tile([128, 4096], BF16)
            nc.gpsimd.dma_start(out=p_t[:, 0:1024], in_=p_d[:, 0:1024])
            nc.sync.dma_start(out=p_t[:, 1024:2048], in_=p_d[:, 1024:2048])
            aux1_t = persist.tile([128, 40], F32)
            nc.gpsimd.dma_start(out=aux1_t, in_=aux1_d[:, :])
            nc.sync.dma_start(out=p_t[:, 2048:3072], in_=p_d[:, 2048:3072])
            aux2_t = persist.tile([128, 256], F32)
            nc.gpsimd.dma_start(out=p_t[:, 3072:4096], in_=p_d[:, 3072:4096])
            nc.sync.dma_start(out=aux2_t, in_=aux2_d[:, :])

            acc_t = persist.tile([128, 64], F32)
            s_t = persist.tile([128, 16], F32)
            fin_sb = persist.tile([2, 32], F32)

            # Main loop. Per (k, quarter): one f32 PSUM tile per image, the
            # two tiles' matmuls interleaved so consecutive MMs target
            # different PE row groups (both sub-array halves run
            # concurrently). relu+bias+accumulate drains each tile on
            # ScalarE or VectorE per act_flags; accumulators land in
            # acc_t[:, 8k+2q+img].
            ti = 0
            for k in range(8):
                b1_ap = aux1_t[:, AUXB1 + k:AUXB1 + k + 1]
                for q in range(4):
                    tiles = []
                    for img in range(2):
                        tiles.append(psum.tile([128, 1024], F32, tag="ps",
                                               bufs=4, name=f"ps_{k}_{q}_{img}"))
                    for j in range(2):
                        for img in range(2):
                            c0 = 1024 * q + 512 * j
                            nc.tensor.matmul(
                                tiles[img][:, 512 * j:512 * j + 512],
                                lhsT=wts_t[64 * img:64 * img + 64,
                                           128 * k:128 * k + 128],
                                rhs=p_t[64 * img:64 * img + 64, c0:c0 + 512],
                                start=True, stop=True,
                            )
                    for img in range(2):
                        col = 8 * k + 2 * q + img
                        accap = acc_t[:, col:col + 1]
                        if act_flags[ti]:
                            nc.scalar.activation(
                                tiles[img], tiles[img],
                                mybir.ActivationFunctionType.Relu,
                                bias=b1_ap, scale=1.0, accum_out=accap,
                            )
                        else:
                            # out = max(in + b1, 0); accum_out = sum(out).
                            # (tensor_scalar would be cheaper but its
                            # accumulator command is LOAD_ACCUMULATE — a
                            # running sum across ops — so the per-tile
                            # sums come out wrong; STT uses
                            # ZERO_ACCUMULATE.)
                            nc.vector.scalar_tensor_tensor(
                                out=tiles[img], in0=tiles[img],
                                scalar=b1_ap,
                                in1=zeros_t.to_broadcast([128, 1024]),
                                op0=mybir.AluOpType.add,
                                op1=mybir.AluOpType.max,
                                accum_out=accap,
                            )
                        ti += 1

            # tail: s[:, 2k+img] = sum_q acc[:, 8k+2q+img], then
            # out[img] = sum_k s_k^T @ M2e_k + b2. A single reduce + 8
            # accumulating matmuls into one borrowed PSUM tile — kept out
            # of the main loop so the relu pipeline never stalls on it.
            nc.vector.tensor_reduce(
                out=s_t.rearrange("p (k i) -> p k i", i=2),
                in_=acc_t.rearrange("p (k q i) -> p k i q", q=4, i=2),
                axis=mybir.AxisListType.X,
                op=mybir.AluOpType.add,
            )
            ps_k = psum.tile([128, 1024], F32, tag="ps", bufs=4,
                             name="ps_fin")
            for k in range(8):
                nc.tensor.matmul(
                    ps_k[0:2, 0:32],
                    lhsT=s_t[:, 2 * k:2 * k + 2],
                    rhs=aux2_t[:, 32 * k:32 * k + 32],
                    start=(k == 0), stop=(k == 7),
                )
            nc.vector.tensor_tensor(
                out=fin_sb, in0=ps_k[0:2, 0:32],
                in1=aux1_t[0:2, AUXB2:AUXB2 + 32],
                op=mybir.AluOpType.add,
            )
            nc.sync.dma_start(out=out_d[:, :], in_=fin_sb)

    nc.compile()
    return nc


_NC_CACHE = None
TRACE = False
_last_result = None
_last_profile_dir = None


def _get_nc():
    global _NC_CACHE
    if _NC_CACHE is None:
        _NC_CACHE = _build_nc()
    return _NC_CACHE


def kernel(x, w1, b1, w2, b2):
    global _last_result
    x = np.ascontiguousarray(np.asarray(x, dtype=np.float32))
    Wt, b1c, M2c, b2t = _build_weights(
        np.asarray(w1, np.float32), np.asarray(b1, np.float32),
        np.asarray(w2, np.float32), np.asarray(b2, np.float32))

    wts = np.empty((128, 1024), dtype=NP_BF16)             # W on both halves
    wts[0:64] = Wt.astype(NP_BF16)
    wts[64:128] = wts[0:64]
    aux1 = np.empty((128, 40), dtype=np.float32)
    aux1[:, AUXB1:AUXB1 + 8] = b1c
    aux1[:, AUXB2:AUXB2 + 32] = b2t
    aux2 = np.ascontiguousarray(M2c)

    # patches: x [16,1,512,512] -> [b, pixel(r,c), patch(i,j)] = [16, 64, 4096]
    p_all = (x.reshape(16, 64, 8, 64, 8).transpose(0, 2, 4, 1, 3)
             .reshape(16, 64, 4096).astype(NP_BF16))

    in_maps = []
    for c in range(N_CORES):
        pc = np.empty((128, 4096), dtype=NP_BF16)
        pc[0:64] = p_all[2 * c]
        pc[64:128] = p_all[2 * c + 1]
        in_maps.append({"p": pc, "wts": wts, "aux1": aux1, "aux2": aux2})

    nc = _get_nc()
    if TRACE:
        # Local profiling path: NTFF via direct ctypes calls into the
        # injected libaxon_pjrt.so (this container's antenv stub lacks
        # axon_hooks, so the registered-hook path is unavailable).
        import contextlib
        import ctypes
        import tempfile
        from concourse import bass2jax

        lib = ctypes.CDLL("/opt/axon/libaxon_pjrt.so")
        lib.axon_start_nrt_profile.argtypes = [
            ctypes.POINTER(ctypes.c_int64), ctypes.c_size_t]
        lib.axon_start_nrt_profile.restype = ctypes.c_int64
        lib.axon_stop_nrt_profile.argtypes = [ctypes.c_char_p]
        lib.axon_stop_nrt_profile.restype = ctypes.c_int64

        @contextlib.contextmanager
        def hook(output_dir, device_ids):
            import jax
            jax.devices()
            if device_ids:
                ids = (ctypes.c_int64 * len(device_ids))(*device_ids)
                rc = lib.axon_start_nrt_profile(ids, len(device_ids))
            else:
                rc = lib.axon_start_nrt_profile(None, 0)
            if rc != 0:
                raise RuntimeError(f"axon_start_nrt_profile rc={rc}")
            try:
                yield
            finally:
                n = lib.axon_stop_nrt_profile(str(output_dir).encode())
                print(f"profile: {n} file(s) written to {output_dir}")

        global _last_profile_dir
        tmpdir = tempfile.mkdtemp(prefix="dctcnn_prof_")
        with hook(tmpdir, [0]):
            results = bass2jax.run_bass_via_pjrt(nc, in_maps, n_cores=N_CORES)
        _last_profile_dir = tmpdir
        out = np.concatenate([results[c]["out"] for c in range(N_CORES)], axis=0)
        return out.astype(np.float32)
    res = run_bass_kernel_spmd(nc, in_maps, list(range(N_CORES)))
    _last_result = res
    out = np.concatenate([res.results[c]["out"] for c in range(N_CORES)], axis=0)
    return out.astype(np.float32)


# revision 25
# speedup vs baseline: 1.2706x; 1.1797x over previous
"""Trainium2 Bass kernel for the DCT-CNN expert core.

Reference computation (per 512x512 single-channel image):
  1. split into 4096 non-overlapping 8x8 patches
  2. 2D DCT per patch:  c = D @ p @ D^T
  3. conv3x3(1->16, SAME) + bias + relu on each 8x8 patch image
  4. conv3x3(16->32, SAME) + bias
  5. mean over spatial (8x8), then mean over patches  -> [B, 32]

Algebraic restructuring (validated to fp32 roundoff):
  - DCT + conv1 fold into one [1024, 64] matrix W = M1 @ (D (x) D); with bias
    b1 broadcast per channel:  h1 = relu(W @ p + b1h)
  - conv2 + spatial mean + patch mean fold into a [1024, 32] matrix applied
    to the per-image SUM of h1: out[b] = (sum_p h1)^T @ M2e + b2

Device schedule per core (2 images = 8192 patches):
  - 64 relu+bias+accumulate tiles of [128, 1024] f32 PSUM (2 banks each,
    3 in flight), split between ScalarE activation and VectorE
    tensor_scalar (both engines read PSUM at 1 elem/cycle/lane; TRN2
    matmul cannot write 16-bit PSUM, so 2x DVE modes are unreachable).
  - PE warmup matmuls on dummy data beat the HAM clock gate before the
    input DMAs land; weights are a single [64, 1024] copy shared by both
    PE row groups via explicit tile_position.
  - final [128,2]x[128,32] matmuls accumulate per-k inside the main loop
    (GPSIMD sums the per-quarter accumulators), leaving a short tail.

Sharding: pure data parallel over images (2 per core), weights replicated.
"""
import numpy as np

import concourse.bass as bass
import concourse.bacc as bacc
import concourse.tile as tile
from concourse import mybir
from concourse.bass_utils import run_bass_kernel_spmd

N_CORES = 8
F32 = mybir.dt.float32
BF16 = mybir.dt.bfloat16

try:
    import ml_dtypes
    NP_BF16 = np.dtype(ml_dtypes.bfloat16)
except ImportError:  # pragma: no cover
    NP_BF16 = None

# ---------------------------------------------------------------- host math

def _dct_matrix(n=8):
    m = np.zeros((n, n), dtype=np.float64)
    for k in range(n):
        for t in range(n):
            if k == 0:
                m[k, t] = 1.0 / np.sqrt(n)
            else:
                m[k, t] = np.sqrt(2.0 / n) * np.cos(np.pi * k * (2 * t + 1) / (2.0 * n))
    return m


def _conv3x3_matrix(w):
    """Dense linear operator of a SAME 3x3 cross-correlation on 8x8 images.

    w: [O, I, 3, 3] -> M: [O*64, I*64] with
    flatten(conv(img))[(o,y,x)] = sum M[(o,y,x),(i,r,c)] img[i,r,c]
    """
    O, I = w.shape[0], w.shape[1]
    M = np.zeros((O, 8, 8, I, 8, 8))
    for dy in range(3):
        for dx in range(3):
            ylo, yhi = max(0, 1 - dy), min(8, 9 - dy)
            xlo, xhi = max(0, 1 - dx), min(8, 9 - dx)
            for y in range(ylo, yhi):
                for x in range(xlo, xhi):
                    M[:, y, x, :, y + dy - 1, x + dx - 1] += w[:, :, dy, dx]
    return M.reshape(O * 64, I * 64)


def _build_weights(w1, b1, w2, b2):
    """Returns (Wt [64,1024], b1c [128,8], M2c [128,256], b2t [128,32]) f32."""
    D = _dct_matrix()
    KRON = np.kron(D, D)                                   # c_flat = KRON @ p_flat
    M1 = _conv3x3_matrix(w1.astype(np.float64))            # [1024, 64]
    M1K = M1 @ KRON                                        # [1024, 64]
    b1h = np.repeat(b1.astype(np.float64), 64)             # [1024]
    M2 = _conv3x3_matrix(w2.astype(np.float64))            # [2048, 1024]
    A2 = M2.reshape(32, 64, 1024).sum(axis=1)              # [32, 1024]
    M2e = A2.T / (64.0 * 4096.0)                           # [1024, 32]

    Wt = np.ascontiguousarray(M1K.T, dtype=np.float32)     # [64, 1024]
    b1c = np.ascontiguousarray(
        b1h.reshape(8, 128).T, dtype=np.float32)           # [128, 8]
    M2c = np.ascontiguousarray(
        M2e.reshape(8, 128, 32).transpose(1, 0, 2).reshape(128, 256),
        dtype=np.float32)                                  # [128, 8*32]
    b2t = np.ascontiguousarray(
        np.tile(b2.astype(np.float32), (128, 1)))          # [128, 32]
    return Wt, b1c, M2c, b2t


# ------------------------------------------------------------- device kernel

# aux1 (f32 columns): [0:8) b1 chunks (col k = b1h[128k:128k+128]);
#                     [8:40) b2 broadcast to all partitions.
# aux2 (f32): M2e chunks (cols 32k..32k+32 = M2e[128k:128k+128, :]).
AUXB1 = 0
AUXB2 = 8

# relu engine assignment: ~32/64 tiles on ScalarE (ACT), rest on VectorE.
_N_TILES = 64
_ACT_SHARE = 32
_N_WARM = 5  # dummy PE warmup matmuls (HAM clock-gate ramp)


def _build_nc():
    nc = bacc.Bacc("TRN2", target_bir_lowering=False, debug=False,
                   num_devices=N_CORES)
    p_d = nc.declare_dram_parameter("p", [128, 4096], BF16, isOutput=False)
    wts_d = nc.declare_dram_parameter("wts", [128, 1024], BF16, isOutput=False)
    aux1_d = nc.declare_dram_parameter("aux1", [128, 40], F32, isOutput=False)
    aux2_d = nc.declare_dram_parameter("aux2", [128, 256], F32, isOutput=False)
    out_d = nc.declare_dram_parameter("out", [2, 32], F32, isOutput=True)

    act_flags = [(((i + 1) * _ACT_SHARE) // _N_TILES) > ((i * _ACT_SHARE) // _N_TILES)
                 for i in range(_N_TILES)]

    with tile.TileContext(nc) as tc:
        with (
            tc.tile_pool(name="persist", bufs=1) as persist,
            tc.tile_pool(name="psum", bufs=1, space="PSUM") as psum,
        ):
            # --- ACT table preload (Relu) on a tiny dummy, ahead of data.
            dum_a = persist.tile([128, 8], F32)
            nc.vector.memset(dum_a, 0.0)
            nc.scalar.activation(dum_a, dum_a,
                                 mybir.ActivationFunctionType.Relu,
                                 bias=0.0, scale=1.0)

            # --- PE warmup: dummy matmuls to ramp the HAM clock gate while
            # the input DMAs are in flight.
            wdum = persist.tile([64, 128], BF16)
            rdum = persist.tile([64, 512], BF16)
            zeros_t = persist.tile([128, 1], F32)
            nc.vector.memset(wdum, 0.0)
            nc.vector.memset(rdum, 0.0)
            nc.vector.memset(zeros_t, 0.0)
            warm_ps = psum.tile([128, 1024], F32, tag="ps", bufs=4,
                                name="warm_ps")
            for _ in range(_N_WARM):
                nc.tensor.matmul(warm_ps[:, 0:512], lhsT=wdum, rhs=rdum,
                                 start=True, stop=True, tile_position=(0, 0))

            # --- input DMAs, ordered so the first matmul's data lands first.
            wts_t = persist.tile([128, 1024], BF16)
            nc.sync.dma_start(out=wts_t, in_=wts_d[:, :])
            p_t = persist.tile([128, 4096], BF16)
            nc.gpsimd.dma_start(out=p_t[:, 0:1024], in_=p_d[:, 0:1024])
            nc.sync.dma_start(out=p_t[:, 1024:2048], in_=p_d[:, 1024:2048])
            aux1_t = persist.tile([128, 40], F32)
            nc.gpsimd.dma_start(out=aux1_t, in_=aux1_d[:, :])
            nc.sync.dma_start(out=p_t[:, 2048:3072], in_=p_d[:, 2048:3072])
            aux2_t = persist.tile([128, 256], F32)
            nc.gpsimd.dma_start(out=p_t[:, 3072:4096], in_=p_d[:, 3072:4096])
            nc.sync.dma_start(out=aux2_t, in_=aux2_d[:, :])

            acc_t = persist.tile([128, 64], F32)
            s_t = persist.tile([128, 16], F32)
            fin_sb = persist.tile([2, 32], F32)

            # Main loop. Per (k, quarter): one f32 PSUM tile per image, the
            # two tiles' matmuls interleaved so consecutive MMs target
            # different PE row groups (both sub-array halves run
            # concurrently). relu+bias+accumulate drains each tile on
            # ScalarE or VectorE per act_flags; accumulators land in
            # acc_t[:, 8k+2q+img].
            ti = 0
            for k in range(8):
                b1_ap = aux1_t[:, AUXB1 + k:AUXB1 + k + 1]
                for q in range(4):
                    tiles = []
                    for img in range(2):
                        tiles.append(psum.tile([128, 1024], F32, tag="ps",
                                               bufs=4, name=f"ps_{k}_{q}_{img}"))
                    for j in range(2):
                        for img in range(2):
                            c0 = 1024 * q + 512 * j
                            nc.tensor.matmul(
                                tiles[img][:, 512 * j:512 * j + 512],
                                lhsT=wts_t[64 * img:64 * img + 64,
                                           128 * k:128 * k + 128],
                                rhs=p_t[64 * img:64 * img + 64, c0:c0 + 512],
                                start=True, stop=True,
                            )
                    for img in range(2):
                        col = 8 * k + 2 * q + img
                        accap = acc_t[:, col:col + 1]
                        if act_flags[ti]:
                            nc.scalar.activation(
                                tiles[img], tiles[img],
                                mybir.ActivationFunctionType.Relu,
                                bias=b1_ap, scale=1.0, accum_out=accap,
                            )
                        else:
                            # out = max(in + b1, 0); accum_out = sum(out).
                            # (tensor_scalar would be cheaper but its
                            # accumulator command is LOAD_ACCUMULATE — a
                            # running sum across ops — so the per-tile
                            # sums come out wrong; STT uses
                            # ZERO_ACCUMULATE.)
                            nc.vector.scalar_tensor_tensor(
                                out=tiles[img], in0=tiles[img],
                                scalar=b1_ap,
                                in1=zeros_t.to_broadcast([128, 1024]),
                                op0=mybir.AluOpType.add,
                                op1=mybir.AluOpType.max,
                                accum_out=accap,
                            )
                        ti += 1

            # tail: s[:, 2k+img] = sum_q acc[:, 8k+2q+img], then
            # out[img] = sum_k s_k^T @ M2e_k + b2. A single reduce + 8
            # accumulating matmuls into one borrowed PSUM tile — kept out
            # of the main loop so the relu pipeline never stalls on it.
            nc.vector.tensor_reduce(
                out=s_t.rearrange("p (k i) -> p k i", i=2),
                in_=acc_t.rearrange("p (k q i) -> p k i q", q=4, i=2),
                axis=mybir.AxisListType.X,
                op=mybir.AluOpType.add,
            )
            ps_k = psum.tile([128, 1024], F32, tag="ps", bufs=4,
                             name="ps_fin")
            for k in range(8):
                nc.tensor.matmul(
                    ps_k[0:2, 0:32],
                    lhsT=s_t[:, 2 * k:2 * k + 2],
                    rhs=aux2_t[:, 32 * k:32 * k + 32],
                    start=(k == 0), stop=(k == 7),
                )
            nc.vector.tensor_tensor(
                out=fin_sb, in0=ps_k[0:2, 0:32],
                in1=aux1_t[0:2, AUXB2:AUXB2 + 32],
                op=mybir.AluOpType.add,
            )
            nc.sync.dma_start(out=out_d[:, :], in_=fin_sb)

    nc.compile()
    return nc


_NC_CACHE = None
TRACE = False
_last_result = None
_last_profile_dir = None


def _get_nc():
    global _NC_CACHE
    if _NC_CACHE is None:
        _NC_CACHE = _build_nc()
    return _NC_CACHE


def kernel(x, w1, b1, w2, b2):
    global _last_result
    x = np.ascontiguousarray(np.asarray(x, dtype=np.float32))
    Wt, b1c, M2c, b2t = _build_weights(
        np.asarray(w1, np.float32), np.asarray(b1, np.float32),
        np.asarray(w2, np.float32), np.asarray(b2, np.float32))

    wts = np.empty((128, 1024), dtype=NP_BF16)             # W on both halves
    wts[0:64] = Wt.astype(NP_BF16)
    wts[64:128] = wts[0:64]
    aux1 = np.empty((128, 40), dtype=np.float32)
    aux1[:, AUXB1:AUXB1 + 8] = b1c
    aux1[:, AUXB2:AUXB2 + 32] = b2t
    aux2 = np.ascontiguousarray(M2c)

    # patches: x [16,1,512,512] -> [b, pixel(r,c), patch(i,j)] = [16, 64, 4096]
    p_all = (x.reshape(16, 64, 8, 64, 8).transpose(0, 2, 4, 1, 3)
             .reshape(16, 64, 4096).astype(NP_BF16))

    in_maps = []
    for c in range(N_CORES):
        pc = np.empty((128, 4096), dtype=NP_BF16)
        pc[0:64] = p_all[2 * c]
        pc[64:128] = p_all[2 * c + 1]
        in_maps.append({"p": pc, "wts": wts, "aux1": aux1, "aux2": aux2})

    nc = _get_nc()
    if TRACE:
        # Local profiling path: NTFF via direct ctypes calls into the
        # injected libaxon_pjrt.so (this container's antenv stub lacks
        # axon_hooks, so the registered-hook path is unavailable).
        import contextlib
        import ctypes
        import tempfile
        from concourse import bass2jax

        lib = ctypes.CDLL("/opt/axon/libaxon_pjrt.so")
        lib.axon_start_nrt_profile.argtypes = [
            ctypes.POINTER(ctypes.c_int64), ctypes.c_size_t]
        lib.axon_start_nrt_profile.restype = ctypes.c_int64
        lib.axon_stop_nrt_profile.argtypes = [ctypes.c_char_p]
        lib.axon_stop_nrt_profile.restype = ctypes.c_int64

        @contextlib.contextmanager
        def hook(output_dir, device_ids):
            import jax
            jax.devices()
            if device_ids:
                ids = (ctypes.c_int64 * len(device_ids))(*device_ids)
                rc = lib.axon_start_nrt_profile(ids, len(device_ids))
            else:
                rc = lib.axon_start_nrt_profile(None, 0)
            if rc != 0:
                raise RuntimeError(f"axon_start_nrt_profile rc={rc}")
            try:
                yield
            finally:
                n = lib.axon_stop_nrt_profile(str(output_dir).encode())
                print(f"profile: {n} file(s) written to {output_dir}")

        global _last_profile_dir
        tmpdir = tempfile.mkdtemp(prefix="dctcnn_prof_")
        with hook(tmpdir, [0]):
            results = bass2jax.run_bass_via_pjrt(nc, in_maps, n_cores=N_CORES)
        _last_profile_dir = tmpdir
        out = np.concatenate([results[c]["out"] for c in range(N_CORES)], axis=0)
        return out.astype(np.float32)
    res = run_bass_kernel_spmd(nc, in_maps, list(range(N_CORES)))
    _last_result = res
    out = np.concatenate([res.results[c]["out"] for c in range(N_CORES)], axis=0)
    return out.astype(np.float32)
